# revision 14
# baseline (speedup 1.0000x reference)
r"""Boson-sampling probability |Perm(A)|^2 via Glynn's formula on 8 Trainium2 cores.

Math
----
perm(A) = 2^(1-n) * sum_{d in {-1,+1}^n} (prod_i d_i) * prod_j (sum_i d_i A_ij), n=20.
Terms for d and -d are equal, so enumerate d_19 = -1 only and double.

Sign-bit allocation for the remaining 19 bits:
  bits 0..8   -> free axis f (512)       [same on every core]
  bits 9..15  -> partition axis p (128)  [same on every core]
  bits 16..18 -> core c (8)

Row vector V_j(p,f,c) = Cp_c[p,j] + Cf[f,j] with
  Cp_c[p,j] = sum_{i=9..15} d_i(p) A[i,j] + sum_{i=16..18} d_i(c) A[i,j] - A[19,j]
  Cf[f,j]   = sum_{i=0..8} d_i(f) A[i,j]

Split the j-product into groups GA=0..6, GB=7..13, GC=14..19. Each group
product expands over subsets T of the group:
  PG[p,f] = sum_T (prod_{j in T} Cp[p,j]) * (prod_{j in G\T} Cf[f,j])
a bilinear form of rank 2^|G| -> computed on TensorE as fp32 matmuls with
PSUM accumulation (contraction over 2*2^|G| re/im-expanded rows). The
per-term parity prod_i d_i is folded into group A's host tables. VectorE
then combines P = PA*PB*PC (complex) and reduces over f with fused
tensor_tensor_reduce ops; the (128,2) per-core partials are summed on host
in float64.
"""

import numpy as np

N = 20
N_CORES = 8
F = 512           # free size (bits 0..8)
P = 128           # partitions (bits 9..15)
GA = list(range(0, 7))
GB = list(range(7, 14))
GC = list(range(14, 20))

_PROGRAM_CACHE = {}


def _signs(count, nbits):
    v = np.arange(count, dtype=np.int64)[:, None]
    return (((v >> np.arange(nbits)) & 1) * 2.0 - 1.0)  # (count, nbits) float64


def _subset_prods(C):
    """C: (nvals, g) complex128 -> (2^g, nvals); row T = prod_{k: bit k of T} C[:, k]."""
    out = np.ones((1, C.shape[0]), np.complex128)
    for k in range(C.shape[1]):
        out = np.concatenate([out, out * C[None, :, k]], axis=0)
    return out


def _pack_group(U, V):
    """Interleave re/im rows for the paired-contraction matmul layout.

    Returns (lhs, rhs_re, rhs_im) with contraction rows m = 2T + comp:
      lhs[2T]   = Re U[T],  lhs[2T+1]  = Im U[T]
      rhs_re[2T]= Re V[T],  rhs_re[2T+1] = -Im V[T]   (-> PG_re)
      rhs_im[2T]= Im V[T],  rhs_im[2T+1] =  Re V[T]   (-> PG_im)
    """
    nT = U.shape[0]
    lhs = np.empty((2 * nT, U.shape[1]), np.float32)
    lhs[0::2] = U.real
    lhs[1::2] = U.imag
    rre = np.empty((2 * nT, V.shape[1]), np.float32)
    rre[0::2] = V.real
    rre[1::2] = -V.imag
    rim = np.empty((2 * nT, V.shape[1]), np.float32)
    rim[0::2] = V.imag
    rim[1::2] = V.real
    return lhs, rre, rim


def _build_core_tables(A, core):
    """Host tables for one core. A: (20,20) complex128."""
    f_signs = _signs(F, 9)
    p_signs = _signs(P, 7)
    c_signs = _signs(N_CORES, 3)
    par_f = np.prod(f_signs, axis=1)
    par_p = np.prod(p_signs, axis=1)
    par_c = np.prod(c_signs[core])

    Cf = f_signs @ A[0:9, :]                                         # (512, 20)
    Cp = p_signs @ A[9:16, :] + (c_signs[core] @ A[16:19, :] - A[19, :])[None, :]

    out = {}
    for name, G in (("A", GA), ("B", GB), ("C", GC)):
        U = _subset_prods(Cp[:, G])          # (2^g, 128)
        VV = _subset_prods(Cf[:, G])         # (2^g, 512)
        V = VV[::-1]                         # complement subset: T -> 2^g-1-T
        if name == "A":
            # fold full parity: par_p(p) * par_f(f) * par_c * (-1 for d19)
            U = U * (par_p[None, :] * (-par_c))
            V = V * par_f[None, :]
        lhs, rre, rim = _pack_group(U, V)
        nchunks = lhs.shape[0] // 128
        packed = np.concatenate([lhs, rre, rim], axis=1)  # (2^g*2, 128+512+512)
        out["tab" + name] = np.ascontiguousarray(
            packed.reshape(nchunks, 128, P + 2 * F))
    return out


def _build_program():
    if "prog" in _PROGRAM_CACHE:
        return _PROGRAM_CACHE["prog"]

    from contextlib import ExitStack
    from concourse import bass, mybir

    f32 = mybir.dt.float32
    mul = mybir.AluOpType.mult
    add = mybir.AluOpType.add
    nc = bass.Bass()

    # DRAM parameters (per-core data is supplied via in_maps; same program on
    # all cores). Each group chunk is one packed tensor [lhsT | rhs_re | rhs_im]
    # so a single DMA gates each pair of fp32 matmuls.
    W = P + 2 * F
    dram = {}
    chunks = []  # (group, k) in DMA order
    for g, nch in (("A", 2), ("B", 2), ("C", 1)):
        dram[g] = nc.declare_dram_parameter("tab" + g, [nch, 128, W], f32, isOutput=False)
        chunks += [(g, k) for k in range(nch)]
    out_dram = nc.declare_dram_parameter("out", [P, 2], f32, isOutput=True)

    es = ExitStack()
    with es:
        block = es.enter_context(nc.Block())
        # one semaphore per load DMA: cross-SDMA-engine completion order is
        # not guaranteed, so thresholds on a shared semaphore would race.
        dsem = [es.enter_context(nc.semaphore(f"dma{i}")) for i in range(len(chunks))]
        pe_sem = es.enter_context(nc.semaphore("pe_sem"))
        dve_sem = es.enter_context(nc.semaphore("dve_sem"))

        sb = {}
        for g, nch in (("A", 2), ("B", 2), ("C", 1)):
            sb[g] = es.enter_context(nc.sbuf_tensor("sb_tab" + g, [128, nch, W], f32))
        names = ["sPAre", "sPAim", "t1", "t2", "t3", "t4", "U_", "W_",
                 "scr1", "scr2", "scr3", "scr4"]
        wt = {n: es.enter_context(nc.sbuf_tensor(n, [P, F], f32)) for n in names}
        racc1 = es.enter_context(nc.sbuf_tensor("racc1", [P, 1], f32))
        racc2 = es.enter_context(nc.sbuf_tensor("racc2", [P, 1], f32))
        racc3 = es.enter_context(nc.sbuf_tensor("racc3", [P, 1], f32))
        racc4 = es.enter_context(nc.sbuf_tensor("racc4", [P, 1], f32))
        out_t = es.enter_context(nc.sbuf_tensor("out_t", [P, 2], f32))
        pg = {}
        for g in ("A", "B", "C"):
            for comp in ("re", "im"):
                pg[g + comp] = es.enter_context(
                    nc.psum_tensor("pg" + g + comp, [P, F], f32))

        @block.sync
        def _(sync):
            for i, (g, k) in enumerate(chunks):
                sync.dma_start(sb[g][:, k, :], dram[g][k]).then_inc(dsem[i], 16)
            sync.wait_ge(dve_sem, 14)
            sync.dma_start(out_dram[:], out_t[:, :]).then_inc(dsem[0], 16)
            sync.wait_ge(dsem[0], 32)

        @block.tensor
        def _(pe):
            # matmul order: Are0 Aim0 | Are1 Aim1 | Bre0 Bim0 | Bre1 Bim1 | Cre Cim
            n_mm = 0
            for i, (g, k) in enumerate(chunks):
                pe.wait_ge(dsem[i], 16)
                nch = 2 if g in ("A", "B") else 1
                for comp, lo in (("re", P), ("im", P + F)):
                    pe.matmul(
                        pg[g + comp][:, :],
                        sb[g][:, k, 0:P],
                        sb[g][:, k, lo:lo + F],
                        start=(k == 0),
                        stop=(k == nch - 1),
                    ).then_inc(pe_sem, 1)
                    n_mm += 1

        @block.vector
        def _(v):
            # pe_sem counts: PA done at 4, PBre at 7, PBim at 8, PCre 9, PCim 10.
            # Every op incs dve_sem; standalone self-waits make same-engine
            # RAW/WAW explicit (TRN2 engines don't guarantee write visibility
            # to the next instruction without a semaphore).
            v.wait_ge(pe_sem, 4)
            v.tensor_copy(wt["sPAre"][:, :], pg["Are"][:, :]).then_inc(dve_sem, 1)
            v.tensor_copy(wt["sPAim"][:, :], pg["Aim"][:, :]).then_inc(dve_sem, 1)
            v.wait_ge(pe_sem, 7)
            v.wait_ge(dve_sem, 2)
            v.tensor_mul(wt["t1"][:, :], wt["sPAre"][:, :], pg["Bre"][:, :]).then_inc(dve_sem, 1)
            v.wait_ge(pe_sem, 8)
            v.tensor_mul(wt["t2"][:, :], wt["sPAim"][:, :], pg["Bim"][:, :]).then_inc(dve_sem, 1)
            v.tensor_mul(wt["t3"][:, :], wt["sPAre"][:, :], pg["Bim"][:, :]).then_inc(dve_sem, 1)
            v.tensor_mul(wt["t4"][:, :], wt["sPAim"][:, :], pg["Bre"][:, :]).then_inc(dve_sem, 1)
            v.wait_ge(dve_sem, 6)
            v.tensor_sub(wt["U_"][:, :], wt["t1"][:, :], wt["t2"][:, :]).then_inc(dve_sem, 1)
            v.tensor_add(wt["W_"][:, :], wt["t3"][:, :], wt["t4"][:, :]).then_inc(dve_sem, 1)
            # re = sum U_*PCre - sum W_*PCim ; im = sum U_*PCim + sum W_*PCre
            # (fused multiply+free-dim-accumulate via scalar_tensor_tensor;
            #  tensor_tensor_reduce doesn't compile in this walrus build)
            v.wait_ge(pe_sem, 10)
            v.wait_ge(dve_sem, 8)
            v.scalar_tensor_tensor(
                wt["scr1"][:, :], wt["U_"][:, :], 1.0, pg["Cre"][:, :],
                mul, mul, accum_out=racc1[:, 0:1]).then_inc(dve_sem, 1)
            v.scalar_tensor_tensor(
                wt["scr2"][:, :], wt["W_"][:, :], 1.0, pg["Cim"][:, :],
                mul, mul, accum_out=racc2[:, 0:1]).then_inc(dve_sem, 1)
            v.scalar_tensor_tensor(
                wt["scr3"][:, :], wt["U_"][:, :], 1.0, pg["Cim"][:, :],
                mul, mul, accum_out=racc3[:, 0:1]).then_inc(dve_sem, 1)
            v.scalar_tensor_tensor(
                wt["scr4"][:, :], wt["W_"][:, :], 1.0, pg["Cre"][:, :],
                mul, mul, accum_out=racc4[:, 0:1]).then_inc(dve_sem, 1)
            v.wait_ge(dve_sem, 10)
            v.tensor_sub(out_t[:, 0:1], racc1[:, 0:1], racc2[:, 0:1]).then_inc(dve_sem, 1)
            v.wait_ge(dve_sem, 12)
            v.tensor_add(out_t[:, 1:2], racc3[:, 0:1], racc4[:, 0:1]).then_inc(dve_sem, 1)

    nc.finalize()
    _PROGRAM_CACHE["prog"] = nc
    return nc


def kernel(A_real, A_imag, _collect=None):
    from concourse.bass_utils import run_bass_kernel_spmd

    A = np.asarray(A_real, np.float64) + 1j * np.asarray(A_imag, np.float64)
    nc = _build_program()
    in_maps = [_build_core_tables(A, c) for c in range(N_CORES)]

    kwargs = dict(_collect or {})
    res = run_bass_kernel_spmd(nc, in_maps, core_ids=list(range(N_CORES)), **kwargs)
    if _collect is not None:
        _collect["results"] = res

    total = np.complex128(0)
    for r in res.results:
        o = np.asarray(r["out"], np.float64)
        total += o[:, 0].sum() + 1j * o[:, 1].sum()

    perm = total * 2.0 * (2.0 ** (1 - N))
    ans = (perm.conjugate() * perm).real
    return np.asarray(ans, np.float32)


# revision 15
# speedup vs baseline: 1.0362x; 1.0362x over previous
r"""Boson-sampling probability |Perm(A)|^2 via Glynn's formula on 8 Trainium2 cores.

Math
----
perm(A) = 2^(1-n) * sum_{d in {-1,+1}^n} (prod_i d_i) * prod_j (sum_i d_i A_ij), n=20.
Terms for d and -d are equal, so enumerate d_19 = -1 only and double.

Sign-bit allocation for the remaining 19 bits:
  bits 0..8   -> free axis f (512)       [same on every core]
  bits 9..15  -> partition axis p (128)  [same on every core]
  bits 16..18 -> core c (8)

Row vector V_j(p,f,c) = Cp_c[p,j] + Cf[f,j] with
  Cp_c[p,j] = sum_{i=9..15} d_i(p) A[i,j] + sum_{i=16..18} d_i(c) A[i,j] - A[19,j]
  Cf[f,j]   = sum_{i=0..8} d_i(f) A[i,j]

Split the j-product into groups GA=0..6, GB=7..13, GC=14..19. Each group
product expands over subsets T of the group:
  PG[p,f] = sum_T (prod_{j in T} Cp[p,j]) * (prod_{j in G\T} Cf[f,j])
a bilinear form of rank 2^|G| -> computed on TensorE as fp32 matmuls with
PSUM accumulation (contraction over 2*2^|G| re/im-expanded rows). The
per-term parity prod_i d_i is folded into group A's host tables. VectorE
then combines P = PA*PB*PC (complex) and reduces over f with fused
tensor_tensor_reduce ops; the (128,2) per-core partials are summed on host
in float64.
"""

import numpy as np

N = 20
N_CORES = 8
F = 512           # free size (bits 0..8)
P = 128           # partitions (bits 9..15)
GA = list(range(0, 7))
GB = list(range(7, 14))
GC = list(range(14, 20))

_PROGRAM_CACHE = {}


def _signs(count, nbits):
    v = np.arange(count, dtype=np.int64)[:, None]
    return (((v >> np.arange(nbits)) & 1) * 2.0 - 1.0)  # (count, nbits) float64


def _subset_prods(C):
    """C: (nvals, g) complex128 -> (2^g, nvals); row T = prod_{k: bit k of T} C[:, k]."""
    out = np.ones((1, C.shape[0]), np.complex128)
    for k in range(C.shape[1]):
        out = np.concatenate([out, out * C[None, :, k]], axis=0)
    return out


def _pack_group(U, V):
    """Interleave re/im rows for the paired-contraction matmul layout.

    Returns (lhs, rhs_re, rhs_im) with contraction rows m = 2T + comp:
      lhs[2T]   = Re U[T],  lhs[2T+1]  = Im U[T]
      rhs_re[2T]= Re V[T],  rhs_re[2T+1] = -Im V[T]   (-> PG_re)
      rhs_im[2T]= Im V[T],  rhs_im[2T+1] =  Re V[T]   (-> PG_im)
    """
    nT = U.shape[0]
    lhs = np.empty((2 * nT, U.shape[1]), np.float32)
    lhs[0::2] = U.real
    lhs[1::2] = U.imag
    rre = np.empty((2 * nT, V.shape[1]), np.float32)
    rre[0::2] = V.real
    rre[1::2] = -V.imag
    rim = np.empty((2 * nT, V.shape[1]), np.float32)
    rim[0::2] = V.imag
    rim[1::2] = V.real
    return lhs, rre, rim


def _build_core_tables(A, core):
    """Host tables for one core. A: (20,20) complex128."""
    f_signs = _signs(F, 9)
    p_signs = _signs(P, 7)
    c_signs = _signs(N_CORES, 3)
    par_f = np.prod(f_signs, axis=1)
    par_p = np.prod(p_signs, axis=1)
    par_c = np.prod(c_signs[core])

    Cf = f_signs @ A[0:9, :]                                         # (512, 20)
    Cp = p_signs @ A[9:16, :] + (c_signs[core] @ A[16:19, :] - A[19, :])[None, :]

    out = {}
    for name, G in (("A", GA), ("B", GB), ("C", GC)):
        U = _subset_prods(Cp[:, G])          # (2^g, 128)
        VV = _subset_prods(Cf[:, G])         # (2^g, 512)
        V = VV[::-1]                         # complement subset: T -> 2^g-1-T
        if name == "A":
            # fold full parity: par_p(p) * par_f(f) * par_c * (-1 for d19)
            U = U * (par_p[None, :] * (-par_c))
            V = V * par_f[None, :]
        lhs, rre, rim = _pack_group(U, V)
        nchunks = lhs.shape[0] // 128
        packed = np.concatenate([lhs, rre, rim], axis=1)  # (2^g*2, 128+512+512)
        out["tab" + name] = np.ascontiguousarray(
            packed.reshape(nchunks, 128, P + 2 * F))
    return out


def _build_program():
    if "prog" in _PROGRAM_CACHE:
        return _PROGRAM_CACHE["prog"]

    from contextlib import ExitStack
    from concourse import bass, mybir

    f32 = mybir.dt.float32
    # FP32R: single-pass PE fp32 path, 4x faster than the 2-pass FP32
    # emulation; measured ~1.4e-4 matmul rel err -> ~1e-3 end-to-end, well
    # inside tolerance. Flip to mybir.dt.float32 for the exact-fp32 fallback.
    mm_dt = mybir.dt.float32r
    mul = mybir.AluOpType.mult
    add = mybir.AluOpType.add
    nc = bass.Bass()

    # DRAM parameters (per-core data is supplied via in_maps; same program on
    # all cores). Each group chunk is one packed tensor [lhsT | rhs_re | rhs_im].
    W = P + 2 * F
    dram = {}
    chunks = []  # (group, k) in DMA order
    for g, nch in (("A", 2), ("B", 2), ("C", 1)):
        dram[g] = nc.declare_dram_parameter("tab" + g, [nch, 128, W], mm_dt, isOutput=False)
        chunks += [(g, k) for k in range(nch)]
    out_dram = nc.declare_dram_parameter("out", [P, 2], f32, isOutput=True)

    es = ExitStack()
    with es:
        block = es.enter_context(nc.Block(no_gpsimd_drain=True))
        # one semaphore per load DMA: cross-SDMA-engine completion order is
        # not guaranteed, so thresholds on a shared semaphore would race.
        dsem = [es.enter_context(nc.semaphore(f"dma{i}")) for i in range(len(chunks))]
        pe_sem = es.enter_context(nc.semaphore("pe_sem"))
        act_sem = es.enter_context(nc.semaphore("act_sem"))
        dve_sem = es.enter_context(nc.semaphore("dve_sem"))

        sb = {}
        for g, nch in (("A", 2), ("B", 2), ("C", 1)):
            sb[g] = es.enter_context(nc.sbuf_tensor("sb_tab" + g, [128, nch, W], mm_dt))
        names = ["sPAre", "sPAim", "t1", "t2", "t3", "t4", "U_", "W_",
                 "scr1", "scr2", "scr3", "scr4"]
        wt = {n: es.enter_context(nc.sbuf_tensor(n, [P, F], f32)) for n in names}
        racc = [es.enter_context(nc.sbuf_tensor(f"racc{i}", [P, 1], f32)) for i in range(4)]
        out_t = es.enter_context(nc.sbuf_tensor("out_t", [P, 2], f32))
        pg = {}
        for g in ("A", "B", "C"):
            for comp in ("re", "im"):
                pg[g + comp] = es.enter_context(
                    nc.psum_tensor("pg" + g + comp, [P, F], f32))

        @block.sync
        def _(sync):
            # A and B chunks on the SP HWDGE ring (A first: it gates everything)
            for i, (g, k) in enumerate(chunks[:4]):
                sync.dma_start(sb[g][:, k, :], dram[g][k]).then_inc(dsem[i], 16)
            sync.wait_ge(dve_sem, 12)
            sync.dma_start(out_dram[:], out_t[:, :]).then_inc(dsem[0], 16)
            sync.wait_ge(dsem[0], 32)

        @block.scalar
        def _(act):
            # C chunk on the ACT HWDGE ring, after A has landed (streams in
            # parallel with B, without stealing bandwidth from A)
            act.wait_ge(dsem[1], 16)
            act.dma_start(sb["C"][:, 0, :], dram["C"][0]).then_inc(dsem[4], 16)
            # PA PSUM->SBUF eviction on ACT so the DVE spends nothing on it
            act.wait_ge(pe_sem, 4)
            act.copy(wt["sPAre"][:, :], pg["Are"][:, :]).then_inc(act_sem, 1)
            act.copy(wt["sPAim"][:, :], pg["Aim"][:, :]).then_inc(act_sem, 1)

        @block.tensor
        def _(pe):
            # matmul order: Are0 Aim0 | Are1 Aim1 | Bre0 Bim0 | Bre1 Bim1 | Cre Cim
            for i, (g, k) in enumerate(chunks):
                pe.wait_ge(dsem[i], 16)
                nch = 2 if g in ("A", "B") else 1
                for comp, lo in (("re", P), ("im", P + F)):
                    pe.matmul(
                        pg[g + comp][:, :],
                        sb[g][:, k, 0:P],
                        sb[g][:, k, lo:lo + F],
                        start=(k == 0),
                        stop=(k == nch - 1),
                    ).then_inc(pe_sem, 1)

        @block.vector
        def _(v):
            # pe_sem counts: PA done at 4, PBre at 7, PBim at 8, PCre 9, PCim 10.
            # Standalone self-waits make same-engine RAW/WAW explicit (TRN2
            # engines don't guarantee write visibility to the next instruction
            # without a semaphore).
            v.wait_ge(pe_sem, 7)
            v.wait_ge(act_sem, 1)
            v.tensor_mul(wt["t1"][:, :], wt["sPAre"][:, :], pg["Bre"][:, :]).then_inc(dve_sem, 1)
            v.wait_ge(pe_sem, 8)
            v.wait_ge(act_sem, 2)
            v.tensor_mul(wt["t2"][:, :], wt["sPAim"][:, :], pg["Bim"][:, :]).then_inc(dve_sem, 1)
            v.tensor_mul(wt["t3"][:, :], wt["sPAre"][:, :], pg["Bim"][:, :]).then_inc(dve_sem, 1)
            v.tensor_mul(wt["t4"][:, :], wt["sPAim"][:, :], pg["Bre"][:, :]).then_inc(dve_sem, 1)
            v.wait_ge(dve_sem, 4)
            v.tensor_sub(wt["U_"][:, :], wt["t1"][:, :], wt["t2"][:, :]).then_inc(dve_sem, 1)
            v.tensor_add(wt["W_"][:, :], wt["t3"][:, :], wt["t4"][:, :]).then_inc(dve_sem, 1)
            # re = sum U_*PCre - sum W_*PCim ; im = sum U_*PCim + sum W_*PCre
            # (fused multiply+free-dim-accumulate via scalar_tensor_tensor;
            #  tensor_tensor_reduce doesn't compile in this walrus build)
            v.wait_ge(pe_sem, 10)
            v.wait_ge(dve_sem, 6)
            v.scalar_tensor_tensor(
                wt["scr1"][:, :], wt["U_"][:, :], 1.0, pg["Cre"][:, :],
                mul, mul, accum_out=racc[0][:, 0:1]).then_inc(dve_sem, 1)
            v.scalar_tensor_tensor(
                wt["scr2"][:, :], wt["W_"][:, :], 1.0, pg["Cim"][:, :],
                mul, mul, accum_out=racc[1][:, 0:1]).then_inc(dve_sem, 1)
            v.scalar_tensor_tensor(
                wt["scr3"][:, :], wt["U_"][:, :], 1.0, pg["Cim"][:, :],
                mul, mul, accum_out=racc[2][:, 0:1]).then_inc(dve_sem, 1)
            v.scalar_tensor_tensor(
                wt["scr4"][:, :], wt["W_"][:, :], 1.0, pg["Cre"][:, :],
                mul, mul, accum_out=racc[3][:, 0:1]).then_inc(dve_sem, 1)
            v.wait_ge(dve_sem, 8)
            v.tensor_sub(out_t[:, 0:1], racc[0][:, 0:1], racc[1][:, 0:1]).then_inc(dve_sem, 1)
            v.wait_ge(dve_sem, 10)
            v.tensor_add(out_t[:, 1:2], racc[2][:, 0:1], racc[3][:, 0:1]).then_inc(dve_sem, 1)

    nc.finalize()
    _PROGRAM_CACHE["prog"] = nc
    return nc


def kernel(A_real, A_imag, _collect=None):
    from concourse.bass_utils import run_bass_kernel_spmd

    A = np.asarray(A_real, np.float64) + 1j * np.asarray(A_imag, np.float64)
    nc = _build_program()
    in_maps = [_build_core_tables(A, c) for c in range(N_CORES)]

    kwargs = dict(_collect or {})
    res = run_bass_kernel_spmd(nc, in_maps, core_ids=list(range(N_CORES)), **kwargs)
    if _collect is not None:
        _collect["results"] = res

    total = np.complex128(0)
    for r in res.results:
        o = np.asarray(r["out"], np.float64)
        total += o[:, 0].sum() + 1j * o[:, 1].sum()

    perm = total * 2.0 * (2.0 ** (1 - N))
    ans = (perm.conjugate() * perm).real
    return np.asarray(ans, np.float32)


# revision 20
# speedup vs baseline: 1.2574x; 1.2134x over previous
r"""Boson-sampling probability |Perm(A)|^2 via Glynn's formula on 8 Trainium2 cores.

Math
----
perm(A) = 2^(1-n) * sum_{d in {-1,+1}^n} (prod_i d_i) * prod_j (sum_i d_i A_ij), n=20.
Terms for d and -d are equal, so enumerate d_19 = -1 only and double.

Sign-bit allocation for the remaining 19 bits:
  bits 0..8   -> free axis f (512)       [same on every core]
  bits 9..15  -> partition axis p (128)  [same on every core]
  bits 16..18 -> core c (8)

Row vector V_j(p,f,c) = Cp_c[p,j] + Cf[f,j] with
  Cp_c[p,j] = sum_{i=9..15} d_i(p) A[i,j] + sum_{i=16..18} d_i(c) A[i,j] - A[19,j]
  Cf[f,j]   = sum_{i=0..8} d_i(f) A[i,j]

Split the j-product into groups GA=0..6, GB=7..13, GC=14..19. Each group
product expands over subsets T of the group:
  PG[p,f] = sum_T (prod_{j in T} Cp[p,j]) * (prod_{j in G\T} Cf[f,j])
a bilinear form of rank 2^|G| -> computed on TensorE as fp32 matmuls with
PSUM accumulation (contraction over 2*2^|G| re/im-expanded rows). The
per-term parity prod_i d_i is folded into group A's host tables. VectorE
then combines P = PA*PB*PC (complex) and reduces over f with fused
tensor_tensor_reduce ops; the (128,2) per-core partials are summed on host
in float64.
"""

import numpy as np

N = 20
N_CORES = 8
F = 512           # free size (bits 0..8)
P = 128           # partitions (bits 9..15)
GA = list(range(0, 7))
GB = list(range(7, 14))
GC = list(range(14, 20))

_PROGRAM_CACHE = {}


def _signs(count, nbits):
    v = np.arange(count, dtype=np.int64)[:, None]
    return (((v >> np.arange(nbits)) & 1) * 2.0 - 1.0)  # (count, nbits) float64


def _subset_prods(C):
    """C: (nvals, g) complex128 -> (2^g, nvals); row T = prod_{k: bit k of T} C[:, k]."""
    out = np.ones((1, C.shape[0]), np.complex128)
    for k in range(C.shape[1]):
        out = np.concatenate([out, out * C[None, :, k]], axis=0)
    return out


def _pack_group(U, V):
    """Interleave re/im rows for the paired-contraction matmul layout.

    One shared V table streams through two matmuls; the re/im arithmetic is
    carried by two lhsT variants (contraction rows m = 2T + c):
      vtab[2T]   = Re V[T],  vtab[2T+1]   = Im V[T]
      lhs_re[2T] = Re U[T],  lhs_re[2T+1] = -Im U[T]   (-> PG_re)
      lhs_im[2T] = Im U[T],  lhs_im[2T+1] =  Re U[T]   (-> PG_im)
    """
    nT = U.shape[0]
    lre = np.empty((2 * nT, U.shape[1]), np.float32)
    lre[0::2] = U.real
    lre[1::2] = -U.imag
    lim = np.empty((2 * nT, U.shape[1]), np.float32)
    lim[0::2] = U.imag
    lim[1::2] = U.real
    vtab = np.empty((2 * nT, V.shape[1]), np.float32)
    vtab[0::2] = V.real
    vtab[1::2] = V.imag
    return lre, lim, vtab


def _build_core_tables(A, core):
    """Host tables for one core. A: (20,20) complex128."""
    f_signs = _signs(F, 9)
    p_signs = _signs(P, 7)
    c_signs = _signs(N_CORES, 3)
    par_f = np.prod(f_signs, axis=1)
    par_p = np.prod(p_signs, axis=1)
    par_c = np.prod(c_signs[core])

    Cf = f_signs @ A[0:9, :]                                         # (512, 20)
    Cp = p_signs @ A[9:16, :] + (c_signs[core] @ A[16:19, :] - A[19, :])[None, :]

    out = {}
    for name, G in (("A", GA), ("B", GB), ("C", GC)):
        U = _subset_prods(Cp[:, G])          # (2^g, 128)
        VV = _subset_prods(Cf[:, G])         # (2^g, 512)
        V = VV[::-1]                         # complement subset: T -> 2^g-1-T
        if name == "A":
            # fold full parity: par_p(p) * par_f(f) * par_c * (-1 for d19)
            U = U * (par_p[None, :] * (-par_c))
            V = V * par_f[None, :]
        lre, lim, vtab = _pack_group(U, V)
        nchunks = lre.shape[0] // 128
        packed = np.concatenate([lre, lim, vtab], axis=1)  # (2^g*2, 128+128+512)
        out["tab" + name] = np.ascontiguousarray(
            packed.reshape(nchunks, 128, 2 * P + F))
    return out


def _build_program():
    if "prog" in _PROGRAM_CACHE:
        return _PROGRAM_CACHE["prog"]

    from contextlib import ExitStack
    from concourse import bass, mybir

    f32 = mybir.dt.float32
    # FP32R: single-pass PE fp32 path, 4x faster than the 2-pass FP32
    # emulation; measured ~1.4e-4 matmul rel err -> ~1e-3 end-to-end, well
    # inside tolerance. Flip to mybir.dt.float32 for the exact-fp32 fallback.
    mm_dt = mybir.dt.float32r
    mul = mybir.AluOpType.mult
    add = mybir.AluOpType.add
    nc = bass.Bass()

    # DRAM parameters (per-core data is supplied via in_maps; same program on
    # all cores). Each group is one packed tensor [lhsT_re | lhsT_im | V].
    W = 2 * P + F
    groups = (("A", 2), ("B", 2), ("C", 1))
    dram = {}
    for g, nch in groups:
        dram[g] = nc.declare_dram_parameter("tab" + g, [nch, 128, W], mm_dt, isOutput=False)
    out_dram = nc.declare_dram_parameter("out", [P, 2], f32, isOutput=True)

    es = ExitStack()
    with es:
        block = es.enter_context(nc.Block(no_gpsimd_drain=True))
        # one semaphore per load DMA: cross-SDMA-engine completion order is
        # not guaranteed, so thresholds on a shared semaphore would race.
        dsem = [es.enter_context(nc.semaphore(f"dma{i}")) for i in range(3)]
        pe_sem = es.enter_context(nc.semaphore("pe_sem"))
        act_sem = es.enter_context(nc.semaphore("act_sem"))
        dve_sem = es.enter_context(nc.semaphore("dve_sem"))

        sb = {}
        for g, nch in groups:
            sb[g] = es.enter_context(nc.sbuf_tensor("sb_tab" + g, [128, nch, W], mm_dt))
        names = ["sPAre", "sPAim", "t1", "t2", "t3", "t4", "U_", "W_",
                 "scr1", "scr2", "scr3", "scr4"]
        wt = {n: es.enter_context(nc.sbuf_tensor(n, [P, F], f32)) for n in names}
        racc = [es.enter_context(nc.sbuf_tensor(f"racc{i}", [P, 1], f32)) for i in range(4)]
        out_t = es.enter_context(nc.sbuf_tensor("out_t", [P, 2], f32))
        pg = {}
        for g in ("A", "B", "C"):
            for comp in ("re", "im"):
                pg[g + comp] = es.enter_context(
                    nc.psum_tensor("pg" + g + comp, [P, F], f32))

        @block.sync
        def _(sync):
            # A and B on the SP HWDGE ring (A first: it gates the ACT copies);
            # one multi-chunk DMA per group.
            for i, (g, nch) in enumerate(groups[:2]):
                sync.dma_start(sb[g][:, :, :], bass.AP(
                    dram[g], 0, [[W, 128], [128 * W, nch], [1, W]])).then_inc(dsem[i], 16)

        @block.scalar
        def _(act):
            # C on the ACT HWDGE ring, after A has landed (streams in
            # parallel with B, without stealing bandwidth from A)
            act.wait_ge(dsem[0], 16)
            act.dma_start(sb["C"][:, 0, :], dram["C"][0]).then_inc(dsem[2], 16)
            # PA PSUM->SBUF eviction on ACT so the DVE spends nothing on it
            act.wait_ge(pe_sem, 4)
            act.copy(wt["sPAre"][:, :], pg["Are"][:, :]).then_inc(act_sem, 1)
            act.copy(wt["sPAim"][:, :], pg["Aim"][:, :]).then_inc(act_sem, 1)
            # store; the end-of-block engine drains cover DMA completion
            act.wait_ge(dve_sem, 12)
            act.dma_start(out_dram[:], out_t[:, :]).then_inc(dsem[2], 16)

        @block.tensor
        def _(pe):
            # matmul order: Are0 Aim0 Are1 Aim1 | Bre0 Bim0 Bre1 Bim1 | Cre Cim
            for i, (g, nch) in enumerate(groups):
                pe.wait_ge(dsem[i], 16)
                for k in range(nch):
                    for comp, lo in (("re", 0), ("im", P)):
                        pe.matmul(
                            pg[g + comp][:, :],
                            sb[g][:, k, lo:lo + P],
                            sb[g][:, k, 2 * P:2 * P + F],
                            start=(k == 0),
                            stop=(k == nch - 1),
                        ).then_inc(pe_sem, 1)

        @block.vector
        def _(v):
            # pe_sem counts: PA done at 4, PBre at 7, PBim at 8, PCre 9, PCim 10.
            # Standalone self-waits make same-engine RAW/WAW explicit (TRN2
            # engines don't guarantee write visibility to the next instruction
            # without a semaphore).
            v.wait_ge(pe_sem, 7)
            v.wait_ge(act_sem, 1)
            v.tensor_mul(wt["t1"][:, :], wt["sPAre"][:, :], pg["Bre"][:, :]).then_inc(dve_sem, 1)
            v.wait_ge(pe_sem, 8)
            v.wait_ge(act_sem, 2)
            v.tensor_mul(wt["t2"][:, :], wt["sPAim"][:, :], pg["Bim"][:, :]).then_inc(dve_sem, 1)
            v.tensor_mul(wt["t3"][:, :], wt["sPAre"][:, :], pg["Bim"][:, :]).then_inc(dve_sem, 1)
            v.tensor_mul(wt["t4"][:, :], wt["sPAim"][:, :], pg["Bre"][:, :]).then_inc(dve_sem, 1)
            v.wait_ge(dve_sem, 4)
            v.tensor_sub(wt["U_"][:, :], wt["t1"][:, :], wt["t2"][:, :]).then_inc(dve_sem, 1)
            v.tensor_add(wt["W_"][:, :], wt["t3"][:, :], wt["t4"][:, :]).then_inc(dve_sem, 1)
            # re = sum U_*PCre - sum W_*PCim ; im = sum U_*PCim + sum W_*PCre
            # (fused multiply+free-dim-accumulate via scalar_tensor_tensor;
            #  tensor_tensor_reduce doesn't compile in this walrus build)
            v.wait_ge(pe_sem, 10)
            v.wait_ge(dve_sem, 6)
            v.scalar_tensor_tensor(
                wt["scr1"][:, :], wt["U_"][:, :], 1.0, pg["Cre"][:, :],
                mul, mul, accum_out=racc[0][:, 0:1]).then_inc(dve_sem, 1)
            v.scalar_tensor_tensor(
                wt["scr2"][:, :], wt["W_"][:, :], 1.0, pg["Cim"][:, :],
                mul, mul, accum_out=racc[1][:, 0:1]).then_inc(dve_sem, 1)
            v.scalar_tensor_tensor(
                wt["scr3"][:, :], wt["U_"][:, :], 1.0, pg["Cim"][:, :],
                mul, mul, accum_out=racc[2][:, 0:1]).then_inc(dve_sem, 1)
            v.scalar_tensor_tensor(
                wt["scr4"][:, :], wt["W_"][:, :], 1.0, pg["Cre"][:, :],
                mul, mul, accum_out=racc[3][:, 0:1]).then_inc(dve_sem, 1)
            v.wait_ge(dve_sem, 8)
            v.tensor_sub(out_t[:, 0:1], racc[0][:, 0:1], racc[1][:, 0:1]).then_inc(dve_sem, 1)
            v.wait_ge(dve_sem, 10)
            v.tensor_add(out_t[:, 1:2], racc[2][:, 0:1], racc[3][:, 0:1]).then_inc(dve_sem, 1)

    nc.finalize()
    _PROGRAM_CACHE["prog"] = nc
    return nc


def kernel(A_real, A_imag, _collect=None):
    from concourse.bass_utils import run_bass_kernel_spmd

    A = np.asarray(A_real, np.float64) + 1j * np.asarray(A_imag, np.float64)
    nc = _build_program()
    in_maps = [_build_core_tables(A, c) for c in range(N_CORES)]

    kwargs = dict(_collect or {})
    res = run_bass_kernel_spmd(nc, in_maps, core_ids=list(range(N_CORES)), **kwargs)
    if _collect is not None:
        _collect["results"] = res

    total = np.complex128(0)
    for r in res.results:
        o = np.asarray(r["out"], np.float64)
        total += o[:, 0].sum() + 1j * o[:, 1].sum()

    perm = total * 2.0 * (2.0 ** (1 - N))
    ans = (perm.conjugate() * perm).real
    return np.asarray(ans, np.float32)


# revision 27
# speedup vs baseline: 1.2870x; 1.0236x over previous
r"""Boson-sampling probability |Perm(A)|^2 via Glynn's formula on 8 Trainium2 cores.

Math
----
perm(A) = 2^(1-n) * sum_{d in {-1,+1}^n} (prod_i d_i) * prod_j (sum_i d_i A_ij), n=20.
Terms for d and -d are equal, so enumerate d_19 = -1 only and double.

Sign-bit allocation for the remaining 19 bits:
  bits 0..8   -> free axis f (512)       [same on every core]
  bits 9..15  -> partition axis p (128)  [same on every core]
  bits 16..18 -> core c (8)

Row vector V_j(p,f,c) = Cp_c[p,j] + Cf[f,j] with
  Cp_c[p,j] = sum_{i=9..15} d_i(p) A[i,j] + sum_{i=16..18} d_i(c) A[i,j] - A[19,j]
  Cf[f,j]   = sum_{i=0..8} d_i(f) A[i,j]

Split the j-product into groups GA=0..6, GB=7..13, GC=14..19. Each group
product expands over subsets T of the group:
  PG[p,f] = sum_T (prod_{j in T} Cp[p,j]) * (prod_{j in G\T} Cf[f,j])
a bilinear form of rank 2^|G| -> computed on TensorE as fp32 matmuls with
PSUM accumulation (contraction over 2*2^|G| re/im-expanded rows). The
per-term parity prod_i d_i is folded into group A's host tables. VectorE
then combines P = PA*PB*PC (complex) and reduces over f with fused
tensor_tensor_reduce ops; the (128,2) per-core partials are summed on host
in float64.
"""

import numpy as np

N = 20
N_CORES = 8
F = 512           # free size (bits 0..8)
P = 128           # partitions (bits 9..15)
GA = list(range(0, 7))
GB = list(range(7, 14))
GC = list(range(14, 20))

_PROGRAM_CACHE = {}


def _signs(count, nbits):
    v = np.arange(count, dtype=np.int64)[:, None]
    return (((v >> np.arange(nbits)) & 1) * 2.0 - 1.0)  # (count, nbits) float64


def _subset_prods(C):
    """C: (nvals, g) complex128 -> (2^g, nvals); row T = prod_{k: bit k of T} C[:, k]."""
    out = np.ones((1, C.shape[0]), np.complex128)
    for k in range(C.shape[1]):
        out = np.concatenate([out, out * C[None, :, k]], axis=0)
    return out


def _pack_group(U, V):
    """Interleave re/im rows for the paired-contraction matmul layout.

    One shared V table streams through two matmuls; the re/im arithmetic is
    carried by two lhsT variants (contraction rows m = 2T + c):
      vtab[2T]   = Re V[T],  vtab[2T+1]   = Im V[T]
      lhs_re[2T] = Re U[T],  lhs_re[2T+1] = -Im U[T]   (-> PG_re)
      lhs_im[2T] = Im U[T],  lhs_im[2T+1] =  Re U[T]   (-> PG_im)
    """
    nT = U.shape[0]
    lre = np.empty((2 * nT, U.shape[1]), np.float32)
    lre[0::2] = U.real
    lre[1::2] = -U.imag
    lim = np.empty((2 * nT, U.shape[1]), np.float32)
    lim[0::2] = U.imag
    lim[1::2] = U.real
    vtab = np.empty((2 * nT, V.shape[1]), np.float32)
    vtab[0::2] = V.real
    vtab[1::2] = V.imag
    return lre, lim, vtab


def _build_core_tables(A, core):
    """Host tables for one core. A: (20,20) complex128."""
    f_signs = _signs(F, 9)
    p_signs = _signs(P, 7)
    c_signs = _signs(N_CORES, 3)
    par_f = np.prod(f_signs, axis=1)
    par_p = np.prod(p_signs, axis=1)
    par_c = np.prod(c_signs[core])

    Cf = f_signs @ A[0:9, :]                                         # (512, 20)
    Cp = p_signs @ A[9:16, :] + (c_signs[core] @ A[16:19, :] - A[19, :])[None, :]

    out = {}
    for name, G in (("A", GA), ("B", GB), ("C", GC)):
        U = _subset_prods(Cp[:, G])          # (2^g, 128)
        VV = _subset_prods(Cf[:, G])         # (2^g, 512)
        V = VV[::-1]                         # complement subset: T -> 2^g-1-T
        if name == "A":
            # fold full parity: par_p(p) * par_f(f) * par_c * (-1 for d19)
            U = U * (par_p[None, :] * (-par_c))
            V = V * par_f[None, :]
        lre, lim, vtab = _pack_group(U, V)
        nchunks = lre.shape[0] // 128
        packed = np.concatenate([lre, lim, vtab], axis=1)  # (2^g*2, 128+128+512)
        out["tab" + name] = np.ascontiguousarray(
            packed.reshape(nchunks, 128, 2 * P + F))
    return out


def _build_program():
    if "prog" in _PROGRAM_CACHE:
        return _PROGRAM_CACHE["prog"]

    from contextlib import ExitStack
    from concourse import bass, mybir

    f32 = mybir.dt.float32
    # FP32R: single-pass PE fp32 path, 4x faster than the 2-pass FP32
    # emulation; measured ~1.4e-4 matmul rel err -> ~1e-3 end-to-end, well
    # inside tolerance. Flip to mybir.dt.float32 for the exact-fp32 fallback.
    mm_dt = mybir.dt.float32r
    mul = mybir.AluOpType.mult
    add = mybir.AluOpType.add
    nc = bass.Bass()

    # DRAM parameters (per-core data is supplied via in_maps; same program on
    # all cores). Each group is one packed tensor [lhsT_re | lhsT_im | V].
    W = 2 * P + F
    groups = (("A", 2), ("B", 2), ("C", 1))
    dram = {}
    for g, nch in groups:
        dram[g] = nc.declare_dram_parameter("tab" + g, [nch, 128, W], mm_dt, isOutput=False)
    out_dram = nc.declare_dram_parameter("out", [P, 2], f32, isOutput=True)

    es = ExitStack()
    with es:
        block = es.enter_context(nc.Block(no_gpsimd_drain=True))
        # one semaphore per load DMA: cross-SDMA-engine completion order is
        # not guaranteed, so thresholds on a shared semaphore would race.
        dsem = [es.enter_context(nc.semaphore(f"dma{i}")) for i in range(5)]
        pe_sem = es.enter_context(nc.semaphore("pe_sem"))
        act_sem = es.enter_context(nc.semaphore("act_sem"))
        dve_sem = es.enter_context(nc.semaphore("dve_sem"))
        gp_sem = es.enter_context(nc.semaphore("gp_sem"))
        warm_sem = es.enter_context(nc.semaphore("warm_sem"))

        sb = {}
        for g, nch in groups:
            sb[g] = es.enter_context(nc.sbuf_tensor("sb_tab" + g, [128, nch, W], mm_dt))
        names = ["sPAre", "sPAim", "sPBre", "sPBim", "t1", "t2", "t3", "t4",
                 "U_", "W_", "scr1", "scr2", "scr3", "scr4"]
        wt = {n: es.enter_context(nc.sbuf_tensor(n, [P, F], f32)) for n in names}
        racc = [es.enter_context(nc.sbuf_tensor(f"racc{i}", [P, 1], f32)) for i in range(4)]
        out_t = es.enter_context(nc.sbuf_tensor("out_t", [P, 2], f32))
        dummy = es.enter_context(nc.sbuf_tensor("actwarm", [P, 2], f32))
        pg = {}
        for g in ("A", "B", "C"):
            for comp in ("re", "im"):
                pg[g + comp] = es.enter_context(
                    nc.psum_tensor("pg" + g + comp, [P, F], f32))

        @block.sync
        def _(sync):
            # per-chunk DMAs on the SP ring: A0 A1 B0 B1 (A first — it gates
            # the ACT evictions; finer chunks let PE start sooner)
            ci = 0
            for g, nch in groups[:2]:
                for k in range(nch):
                    sync.dma_start(sb[g][:, k, :], dram[g][k]).then_inc(dsem[ci], 16)
                    ci += 1

        @block.scalar
        def _(act):
            # touch ACT once before any gating wait so walrus's activation
            # table load happens during the DMA head, off the critical path
            act.wait_ge(warm_sem, 1)
            act.copy(dummy[:, 1:2], dummy[:, 0:1])
            # C on the ACT HWDGE ring, after A has landed (streams in
            # parallel with B, without stealing bandwidth from A)
            act.wait_ge(dsem[1], 16)
            act.dma_start(sb["C"][:, 0, :], dram["C"][0]).then_inc(dsem[4], 16)
            # PSUM->SBUF evictions of PA and PB so DVE/GpSimd combine ops
            # have at most one PSUM operand (and GpSimd, which cannot touch
            # PSUM, can help at all)
            act.wait_ge(pe_sem, 4)
            act.copy(wt["sPAre"][:, :], pg["Are"][:, :]).then_inc(act_sem, 1)
            act.copy(wt["sPAim"][:, :], pg["Aim"][:, :]).then_inc(act_sem, 1)
            # store; the end-of-block engine drains cover DMA completion
            act.wait_ge(dve_sem, 12)
            act.dma_start(out_dram[:], out_t[:, :]).then_inc(dsem[4], 16)

        @block.tensor
        def _(pe):
            # matmul order: Are0 Aim0 Are1 Aim1 | Bre0 Bim0 Bre1 Bim1 | Cre Cim
            ci = 0
            for gi, (g, nch) in enumerate(groups):
                for k in range(nch):
                    pe.wait_ge(dsem[ci], 16)
                    ci += 1
                    for comp, lo in (("re", 0), ("im", P)):
                        pe.matmul(
                            pg[g + comp][:, :],
                            sb[g][:, k, lo:lo + P],
                            sb[g][:, k, 2 * P:2 * P + F],
                            start=(k == 0),
                            stop=(k == nch - 1),
                        ).then_inc(pe_sem, 1)

        @block.vector
        def _(v):
            # pe_sem counts: PA done at 4, PBre at 7, PBim at 8, PCre 9, PCim 10.
            # Standalone self-waits make same-engine RAW/WAW explicit.
            v.memset(dummy[:, 0:1], 0.0).then_inc(warm_sem, 1)
            v.wait_ge(pe_sem, 7)
            v.wait_ge(act_sem, 1)
            v.tensor_mul(wt["t1"][:, :], wt["sPAre"][:, :], pg["Bre"][:, :]).then_inc(dve_sem, 1)
            v.wait_ge(pe_sem, 8)
            v.wait_ge(act_sem, 2)
            v.tensor_mul(wt["t2"][:, :], wt["sPAim"][:, :], pg["Bim"][:, :]).then_inc(dve_sem, 1)
            v.tensor_mul(wt["t3"][:, :], wt["sPAre"][:, :], pg["Bim"][:, :]).then_inc(dve_sem, 1)
            v.tensor_mul(wt["t4"][:, :], wt["sPAim"][:, :], pg["Bre"][:, :]).then_inc(dve_sem, 1)
            v.wait_ge(dve_sem, 4)
            v.tensor_sub(wt["U_"][:, :], wt["t1"][:, :], wt["t2"][:, :]).then_inc(dve_sem, 1)
            v.tensor_add(wt["W_"][:, :], wt["t3"][:, :], wt["t4"][:, :]).then_inc(dve_sem, 1)
            # re = sum U_*PCre - sum W_*PCim ; im = sum U_*PCim + sum W_*PCre
            v.wait_ge(pe_sem, 10)
            v.wait_ge(dve_sem, 6)
            v.scalar_tensor_tensor(
                wt["scr1"][:, :], wt["U_"][:, :], 1.0, pg["Cre"][:, :],
                mul, mul, accum_out=racc[0][:, 0:1]).then_inc(dve_sem, 1)
            v.scalar_tensor_tensor(
                wt["scr2"][:, :], wt["W_"][:, :], 1.0, pg["Cim"][:, :],
                mul, mul, accum_out=racc[1][:, 0:1]).then_inc(dve_sem, 1)
            v.scalar_tensor_tensor(
                wt["scr3"][:, :], wt["U_"][:, :], 1.0, pg["Cim"][:, :],
                mul, mul, accum_out=racc[2][:, 0:1]).then_inc(dve_sem, 1)
            v.scalar_tensor_tensor(
                wt["scr4"][:, :], wt["W_"][:, :], 1.0, pg["Cre"][:, :],
                mul, mul, accum_out=racc[3][:, 0:1]).then_inc(dve_sem, 1)
            v.wait_ge(dve_sem, 10)
            v.tensor_sub(out_t[:, 0:1], racc[0][:, 0:1], racc[1][:, 0:1]).then_inc(dve_sem, 1)
            v.tensor_add(out_t[:, 1:2], racc[2][:, 0:1], racc[3][:, 0:1]).then_inc(dve_sem, 1)

    nc.finalize()
    _PROGRAM_CACHE["prog"] = nc
    return nc


def kernel(A_real, A_imag, _collect=None):
    from concourse.bass_utils import run_bass_kernel_spmd

    A = np.asarray(A_real, np.float64) + 1j * np.asarray(A_imag, np.float64)
    nc = _build_program()
    in_maps = [_build_core_tables(A, c) for c in range(N_CORES)]

    kwargs = dict(_collect or {})
    res = run_bass_kernel_spmd(nc, in_maps, core_ids=list(range(N_CORES)), **kwargs)
    if _collect is not None:
        _collect["results"] = res

    total = np.complex128(0)
    for r in res.results:
        o = np.asarray(r["out"], np.float64)
        total += o[:, 0].sum() + 1j * o[:, 1].sum()

    perm = total * 2.0 * (2.0 ** (1 - N))
    ans = (perm.conjugate() * perm).real
    return np.asarray(ans, np.float32)


# revision 28
# speedup vs baseline: 1.2930x; 1.0047x over previous
r"""Boson-sampling probability |Perm(A)|^2 via Glynn's formula on 8 Trainium2 cores.

Math
----
perm(A) = 2^(1-n) * sum_{d in {-1,+1}^n} (prod_i d_i) * prod_j (sum_i d_i A_ij), n=20.
Terms for d and -d are equal, so enumerate d_19 = -1 only and double.

Sign-bit allocation for the remaining 19 bits:
  bits 0..8   -> free axis f (512)       [same on every core]
  bits 9..15  -> partition axis p (128)  [same on every core]
  bits 16..18 -> core c (8)

Row vector V_j(p,f,c) = Cp_c[p,j] + Cf[f,j] with
  Cp_c[p,j] = sum_{i=9..15} d_i(p) A[i,j] + sum_{i=16..18} d_i(c) A[i,j] - A[19,j]
  Cf[f,j]   = sum_{i=0..8} d_i(f) A[i,j]

Split the j-product into groups GA=0..6, GB=7..13, GC=14..19. Each group
product expands over subsets T of the group:
  PG[p,f] = sum_T (prod_{j in T} Cp[p,j]) * (prod_{j in G\T} Cf[f,j])
a bilinear form of rank 2^|G| -> computed on TensorE as fp32 matmuls with
PSUM accumulation (contraction over 2*2^|G| re/im-expanded rows). The
per-term parity prod_i d_i is folded into group A's host tables. VectorE
then combines P = PA*PB*PC (complex) and reduces over f with fused
tensor_tensor_reduce ops; the (128,2) per-core partials are summed on host
in float64.
"""

import numpy as np

N = 20
N_CORES = 8
F = 512           # free size (bits 0..8)
P = 128           # partitions (bits 9..15)
GA = list(range(0, 7))
GB = list(range(7, 14))
GC = list(range(14, 20))

_PROGRAM_CACHE = {}


def _signs(count, nbits):
    v = np.arange(count, dtype=np.int64)[:, None]
    return (((v >> np.arange(nbits)) & 1) * 2.0 - 1.0)  # (count, nbits) float64


def _subset_prods(C):
    """C: (nvals, g) complex128 -> (2^g, nvals); row T = prod_{k: bit k of T} C[:, k]."""
    out = np.ones((1, C.shape[0]), np.complex128)
    for k in range(C.shape[1]):
        out = np.concatenate([out, out * C[None, :, k]], axis=0)
    return out


def _pack_group(U, V):
    """Interleave re/im rows for the paired-contraction matmul layout.

    One shared V table streams through two matmuls; the re/im arithmetic is
    carried by two lhsT variants (contraction rows m = 2T + c):
      vtab[2T]   = Re V[T],  vtab[2T+1]   = Im V[T]
      lhs_re[2T] = Re U[T],  lhs_re[2T+1] = -Im U[T]   (-> PG_re)
      lhs_im[2T] = Im U[T],  lhs_im[2T+1] =  Re U[T]   (-> PG_im)
    """
    nT = U.shape[0]
    lre = np.empty((2 * nT, U.shape[1]), np.float32)
    lre[0::2] = U.real
    lre[1::2] = -U.imag
    lim = np.empty((2 * nT, U.shape[1]), np.float32)
    lim[0::2] = U.imag
    lim[1::2] = U.real
    vtab = np.empty((2 * nT, V.shape[1]), np.float32)
    vtab[0::2] = V.real
    vtab[1::2] = V.imag
    return lre, lim, vtab


def _build_core_tables(A, core):
    """Host tables for one core. A: (20,20) complex128."""
    f_signs = _signs(F, 9)
    p_signs = _signs(P, 7)
    c_signs = _signs(N_CORES, 3)
    par_f = np.prod(f_signs, axis=1)
    par_p = np.prod(p_signs, axis=1)
    par_c = np.prod(c_signs[core])

    Cf = f_signs @ A[0:9, :]                                         # (512, 20)
    Cp = p_signs @ A[9:16, :] + (c_signs[core] @ A[16:19, :] - A[19, :])[None, :]

    out = {}
    for name, G in (("A", GA), ("B", GB), ("C", GC)):
        U = _subset_prods(Cp[:, G])          # (2^g, 128)
        VV = _subset_prods(Cf[:, G])         # (2^g, 512)
        V = VV[::-1]                         # complement subset: T -> 2^g-1-T
        if name == "A":
            # fold full parity: par_p(p) * par_f(f) * par_c * (-1 for d19)
            U = U * (par_p[None, :] * (-par_c))
            V = V * par_f[None, :]
        lre, lim, vtab = _pack_group(U, V)
        nchunks = lre.shape[0] // 128
        packed = np.concatenate([lre, lim, vtab], axis=1)  # (2^g*2, 128+128+512)
        out["tab" + name] = np.ascontiguousarray(
            packed.reshape(nchunks, 128, 2 * P + F))
    return out


def _build_program():
    if "prog" in _PROGRAM_CACHE:
        return _PROGRAM_CACHE["prog"]

    from contextlib import ExitStack
    from concourse import bass, mybir

    f32 = mybir.dt.float32
    # FP32R: single-pass PE fp32 path, 4x faster than the 2-pass FP32
    # emulation; measured ~1.4e-4 matmul rel err -> ~1e-3 end-to-end, well
    # inside tolerance. Flip to mybir.dt.float32 for the exact-fp32 fallback.
    mm_dt = mybir.dt.float32r
    mul = mybir.AluOpType.mult
    add = mybir.AluOpType.add
    nc = bass.Bass()

    # DRAM parameters (per-core data is supplied via in_maps; same program on
    # all cores). Each group is one packed tensor [lhsT_re | lhsT_im | V].
    W = 2 * P + F
    groups = (("A", 2), ("B", 2), ("C", 1))
    dram = {}
    for g, nch in groups:
        dram[g] = nc.declare_dram_parameter("tab" + g, [nch, 128, W], mm_dt, isOutput=False)
    out_dram = nc.declare_dram_parameter("out", [P, 2], f32, isOutput=True)

    es = ExitStack()
    with es:
        block = es.enter_context(nc.Block(no_gpsimd_drain=True))
        # one semaphore per load DMA: cross-SDMA-engine completion order is
        # not guaranteed, so thresholds on a shared semaphore would race.
        dsem = [es.enter_context(nc.semaphore(f"dma{i}")) for i in range(5)]
        pe_sem = es.enter_context(nc.semaphore("pe_sem"))
        act_sem = es.enter_context(nc.semaphore("act_sem"))
        dve_sem = es.enter_context(nc.semaphore("dve_sem"))
        gp_sem = es.enter_context(nc.semaphore("gp_sem"))
        warm_sem = es.enter_context(nc.semaphore("warm_sem"))

        sb = {}
        for g, nch in groups:
            sb[g] = es.enter_context(nc.sbuf_tensor("sb_tab" + g, [128, nch, W], mm_dt))
        names = ["sPCre", "sPCim", "t1", "t2", "t3", "t4",
                 "U_", "W_", "scr1", "scr2", "scr3", "scr4"]
        wt = {n: es.enter_context(nc.sbuf_tensor(n, [P, F], f32)) for n in names}
        racc = [es.enter_context(nc.sbuf_tensor(f"racc{i}", [P, 1], f32)) for i in range(4)]
        out_t = es.enter_context(nc.sbuf_tensor("out_t", [P, 2], f32))
        dummy = es.enter_context(nc.sbuf_tensor("actwarm", [P, 2], f32))
        pg = {}
        for g in ("A", "B", "C"):
            for comp in ("re", "im"):
                pg[g + comp] = es.enter_context(
                    nc.psum_tensor("pg" + g + comp, [P, F], f32))

        # DMA order = consumption order: C gates the ACT evictions (first),
        # B gates the DVE M-stage, A gates only the final fused accumulates
        # (lands last). All five loads go FIFO down the SP HWDGE ring.
        chunk_list = [("C", 0), ("B", 0), ("B", 1), ("A", 0), ("A", 1)]

        @block.sync
        def _(sync):
            for i, (g, k) in enumerate(chunk_list):
                sync.dma_start(sb[g][:, k, :], dram[g][k]).then_inc(dsem[i], 16)

        @block.scalar
        def _(act):
            # touch ACT once before any gating wait so walrus's activation
            # table load happens during the DMA head, off the critical path
            act.wait_ge(warm_sem, 1)
            act.copy(dummy[:, 1:2], dummy[:, 0:1])
            # PSUM->SBUF eviction of PC so DVE combine ops have at most one
            # PSUM operand
            act.wait_ge(pe_sem, 2)
            act.copy(wt["sPCre"][:, :], pg["Cre"][:, :]).then_inc(act_sem, 1)
            act.copy(wt["sPCim"][:, :], pg["Cim"][:, :]).then_inc(act_sem, 1)
            # store; the end-of-block engine drains cover DMA completion
            act.wait_ge(dve_sem, 12)
            act.dma_start(out_dram[:], out_t[:, :]).then_inc(dsem[0], 16)

        @block.tensor
        def _(pe):
            # matmul order: Cre Cim | Bre0 Bim0 Bre1 Bim1 | Are0 Aim0 Are1 Aim1
            # pe_sem: PC done at 2, PBre at 5, PBim at 6, PAre at 9, PAim at 10
            for i, (g, k) in enumerate(chunk_list):
                pe.wait_ge(dsem[i], 16)
                nch = 2 if g in ("A", "B") else 1
                for comp, lo in (("re", 0), ("im", P)):
                    pe.matmul(
                        pg[g + comp][:, :],
                        sb[g][:, k, lo:lo + P],
                        sb[g][:, k, 2 * P:2 * P + F],
                        start=(k == 0),
                        stop=(k == nch - 1),
                    ).then_inc(pe_sem, 1)

        @block.vector
        def _(v):
            # M = PC*PB on DVE; standalone self-waits make same-engine
            # RAW/WAW explicit.
            v.memset(dummy[:, 0:1], 0.0).then_inc(warm_sem, 1)
            v.wait_ge(pe_sem, 5)
            v.wait_ge(act_sem, 1)
            v.tensor_mul(wt["t1"][:, :], wt["sPCre"][:, :], pg["Bre"][:, :]).then_inc(dve_sem, 1)
            v.wait_ge(pe_sem, 6)
            v.wait_ge(act_sem, 2)
            v.tensor_mul(wt["t2"][:, :], wt["sPCim"][:, :], pg["Bim"][:, :]).then_inc(dve_sem, 1)
            v.tensor_mul(wt["t3"][:, :], wt["sPCre"][:, :], pg["Bim"][:, :]).then_inc(dve_sem, 1)
            v.tensor_mul(wt["t4"][:, :], wt["sPCim"][:, :], pg["Bre"][:, :]).then_inc(dve_sem, 1)
            v.wait_ge(dve_sem, 4)
            v.tensor_sub(wt["U_"][:, :], wt["t1"][:, :], wt["t2"][:, :]).then_inc(dve_sem, 1)
            v.tensor_add(wt["W_"][:, :], wt["t3"][:, :], wt["t4"][:, :]).then_inc(dve_sem, 1)
            # re = sum U_*PAre - sum W_*PAim ; im = sum U_*PAim + sum W_*PAre
            v.wait_ge(pe_sem, 9)
            v.wait_ge(dve_sem, 6)
            v.scalar_tensor_tensor(
                wt["scr1"][:, :], wt["U_"][:, :], 1.0, pg["Are"][:, :],
                mul, mul, accum_out=racc[0][:, 0:1]).then_inc(dve_sem, 1)
            v.wait_ge(pe_sem, 10)
            v.scalar_tensor_tensor(
                wt["scr2"][:, :], wt["W_"][:, :], 1.0, pg["Aim"][:, :],
                mul, mul, accum_out=racc[1][:, 0:1]).then_inc(dve_sem, 1)
            v.scalar_tensor_tensor(
                wt["scr3"][:, :], wt["U_"][:, :], 1.0, pg["Aim"][:, :],
                mul, mul, accum_out=racc[2][:, 0:1]).then_inc(dve_sem, 1)
            v.scalar_tensor_tensor(
                wt["scr4"][:, :], wt["W_"][:, :], 1.0, pg["Are"][:, :],
                mul, mul, accum_out=racc[3][:, 0:1]).then_inc(dve_sem, 1)
            v.wait_ge(dve_sem, 10)
            v.tensor_sub(out_t[:, 0:1], racc[0][:, 0:1], racc[1][:, 0:1]).then_inc(dve_sem, 1)
            v.tensor_add(out_t[:, 1:2], racc[2][:, 0:1], racc[3][:, 0:1]).then_inc(dve_sem, 1)

    nc.finalize()
    _PROGRAM_CACHE["prog"] = nc
    return nc


def kernel(A_real, A_imag, _collect=None):
    from concourse.bass_utils import run_bass_kernel_spmd

    A = np.asarray(A_real, np.float64) + 1j * np.asarray(A_imag, np.float64)
    nc = _build_program()
    in_maps = [_build_core_tables(A, c) for c in range(N_CORES)]

    kwargs = dict(_collect or {})
    res = run_bass_kernel_spmd(nc, in_maps, core_ids=list(range(N_CORES)), **kwargs)
    if _collect is not None:
        _collect["results"] = res

    total = np.complex128(0)
    for r in res.results:
        o = np.asarray(r["out"], np.float64)
        total += o[:, 0].sum() + 1j * o[:, 1].sum()

    perm = total * 2.0 * (2.0 ** (1 - N))
    ans = (perm.conjugate() * perm).real
    return np.asarray(ans, np.float32)
